# revision 31
# baseline (speedup 1.0000x reference)
"""Trainium2 Bass kernel: full 1-D convolution (2,097,152-sample signal with a
32,000-tap RIR) followed by peak-normalization, distributed over 8 NeuronCores.

Algorithm: block-Toeplitz formulation of the convolution.
  y[128q + r] = sum_c sum_s  h[128c + r - s] * x_{q-c}[s]
i.e. y_q = sum_{c=0}^{250} M_c^T x_{q-c}, where M_c[s, r] = h[128c + r - s]
(zero outside [0, 32000)).  Each M_c is a 128x128 Toeplitz matrix built on the
host from the RIR; the x blocks live as columns of an SBUF-resident [128, nblk]
matrix, so the tensor engine runs 251 accumulating matmuls per output tile with
the rhs being a sliding column window of the same SBUF tensor.

Sequence-parallel across 8 cores: core i computes output blocks
[2080 i, 2080 (i+1)) and receives its input span plus a 250-block left halo.
The peak-normalize max is combined with an on-device AllReduce(max) and the
scale (1 / max(m, 1)) is applied before the output leaves the device.

Matmuls run in float32r (TF32-like, full bf16-rate on the PE at free dim >=
256) with fp32 PSUM accumulation.
"""
import numpy as np

B = 128                      # block size / partition count
KLEN = 32_000                # RIR taps
N = 2_097_152                # signal samples
NOUT = N + KLEN - 1          # full-convolution output length
C = 251                      # number of 128-tap Toeplitz chunks
HALO = C - 1                 # left halo in blocks
NCORES = 8
BLK_PER_CORE = 2080          # output blocks per core (8*2080 = 16640 >= 16634)
LOCAL_IN = BLK_PER_CORE + HALO   # input blocks per core (2330)
F = 416                      # matmul moving free dim (psum tile columns)
NTILES = BLK_PER_CORE // F   # 5 psum tiles per core

_NC_CACHE = None


def _build_nc(collective=True, passes=1):
    import concourse.bacc as bacc
    import concourse.bass as bass
    import concourse.mybir as mybir
    from concourse import tile

    f32 = mybir.dt.float32
    f32r = mybir.dt.float32r

    nc = bacc.Bacc("TRN2", target_bir_lowering=False, debug=False,
                   num_devices=NCORES)

    x_in = nc.dram_tensor("x", [B, LOCAL_IN], f32r, kind="ExternalInput")
    w_in = nc.dram_tensor("w", [B, C * B], f32r, kind="ExternalInput")
    y_out = nc.dram_tensor("y", [B, BLK_PER_CORE], f32, kind="ExternalOutput")

    with tile.TileContext(nc) as tc:
        with (
            tc.tile_pool(name="data", bufs=1) as data_pool,
            tc.tile_pool(name="wpool", bufs=1) as wpool,
            tc.tile_pool(name="ps", bufs=1, space="PSUM") as ps_pool,
            tc.tile_pool(name="dram", bufs=1, space="DRAM") as dram_pool,
        ):
            x_sb = data_pool.tile([B, LOCAL_IN], f32r, name="x_sb")
            y_sb = data_pool.tile([B, BLK_PER_CORE], f32, name="y_sb")
            mx = data_pool.tile([B, NTILES], f32, name="mx")
            w_sb = wpool.tile([B, C * B], f32r, name="w_sb")

            # warm the PE (HAM clock gate) during the initial DMA wait with
            # dummy matmuls on a zeroed tile — no data dependencies
            warm = data_pool.tile([B, 512], mybir.dt.bfloat16, name="warm")
            nc.gpsimd.memset(warm[:], 0.0)
            wps = ps_pool.tile([B, 512], f32, name="wps", tag="wps")
            for _ in range(12):
                nc.tensor.matmul(wps[:], warm[:, :B], warm[:, :512],
                                 start=True, stop=True)

            # tile 0 reads x cols [0, HALO+F); load those first (on a separate
            # engine queue from the weights) so the PE can start early.
            XSPLIT = HALO + F
            nc.scalar.dma_start(x_sb[:, :XSPLIT], x_in[:, :XSPLIT])
            # weights in batches, in consumption order (c ascending), small
            # batches first so chunk delivery keeps pace with the tile-0
            # c-loop from the start
            ws = [4, 8, 16]
            while sum(ws) < C:
                ws.append(min(16, C - sum(ws)))
            b0 = 0
            for i, wn in enumerate(ws):
                b1 = b0 + wn
                nc.sync.dma_start(w_sb[:, b0 * B:b1 * B], w_in[:, b0 * B:b1 * B])
                if i == 0:
                    nc.scalar.dma_start(x_sb[:, XSPLIT:], x_in[:, XSPLIT:])
                b0 = b1

            def one_pass():
                # chunk-major: all NTILES psum tiles accumulate concurrently
                # (separate banks), so each weight chunk is consumed NTILES
                # matmuls in a row — chunk delivery from HBM (~178 ns/chunk)
                # can never stall the PE (~5x134 ns of work per chunk)
                pss = [ps_pool.tile([B, F], f32, name=f"ps{t}", tag=f"ps{t}")
                       for t in range(NTILES)]
                for c in range(C):
                    for t in range(NTILES):
                        lo = t * F + HALO - c
                        nc.tensor.matmul(
                            pss[t][:],
                            w_sb[:, c * B:(c + 1) * B],
                            x_sb[:, lo:lo + F],
                            start=(c == 0),
                            stop=(c == C - 1),
                        )
                for t in range(NTILES):
                    nc.vector.tensor_reduce(
                        mx[:, t:t + 1], pss[t][:], axis=mybir.AxisListType.X,
                        op=mybir.AluOpType.max, apply_absolute_value=True,
                    )

                # local scalar max -> all partitions
                am = data_pool.tile([B, 1], f32, name="am")
                nc.vector.tensor_reduce(
                    am[:], mx[:], axis=mybir.AxisListType.X,
                    op=mybir.AluOpType.max,
                )
                gm = data_pool.tile([B, 1], f32, name="gm")
                nc.gpsimd.partition_all_reduce(
                    gm[:], am[:], B, bass.bass_isa.ReduceOp.max
                )

                scb = data_pool.tile([B, 1], f32, name="scb")
                if collective:
                    # global max across the 8 cores
                    cc_in = dram_pool.tile([B, 1], f32, name="cc_in")
                    cc_out = dram_pool.tile([B, 1], f32, name="cc_out",
                                            addr_space="Shared")
                    nc.sync.dma_start(cc_in[:], gm[:])
                    nc.gpsimd.collective_compute(
                        "AllReduce",
                        mybir.AluOpType.max,
                        replica_groups=[list(range(NCORES))],
                        ins=[cc_in[:].opt()],
                        outs=[cc_out[:].opt()],
                    )
                    nc.sync.dma_start(scb[:], cc_out[:])
                else:
                    # single-core variant for TimelineSim (no collectives)
                    nc.vector.tensor_copy(scb[:], gm[:])

                # y *= 1 / max(m, 1); chunked so the store DMAs (on two
                # queues) overlap the scaling
                nc.vector.tensor_scalar_max(scb[:], scb[:], 1.0)
                nc.vector.reciprocal(scb[:], scb[:])
                for t in range(NTILES):
                    sl = slice(t * F, (t + 1) * F)
                    nc.vector.tensor_scalar_mul(y_sb[:, sl], pss[t][:],
                                                scb[:, 0:1])
                    eng = nc.sync if t % 2 == 0 else nc.scalar
                    eng.dma_start(y_out[:, sl], y_sb[:, sl])

            for _ in range(passes):  # passes > 1 only for wall-clock timing
                one_pass()

    nc.compile()
    return nc


def _build_weights(h):
    """[128, C*128] f32: column block c holds M_c with M_c[s, r] = h[128c+r-s]."""
    hp = np.zeros(B * (C - 1) + 2 * B, np.float32)
    hp[B - 1:B - 1 + KLEN] = h
    idx = (B - 1) + np.arange(B)[None, :] - np.arange(B)[:, None]  # [s, r]
    offs = B * np.arange(C)
    W = hp[offs[:, None, None] + idx[None, :, :]]                  # [C, s, r]
    return np.ascontiguousarray(W.transpose(1, 0, 2).reshape(B, C * B))


def _shard(data, i):
    """Core i's input: blocks [q0-HALO, q0+BLK_PER_CORE) as a [128, LOCAL_IN]
    matrix (column q = block q), zero-padded outside [0, N)."""
    q0 = i * BLK_PER_CORE
    lo = (q0 - HALO) * B
    hi = (q0 + BLK_PER_CORE) * B
    seg = np.zeros(hi - lo, np.float32)
    s0, s1 = max(lo, 0), min(hi, N)
    if s1 > s0:
        seg[s0 - lo:s1 - lo] = data[s0:s1]
    return np.ascontiguousarray(seg.reshape(LOCAL_IN, B).T)


def kernel(data, rir):
    global _NC_CACHE
    from concourse.bass_utils import run_bass_kernel_spmd

    data = np.asarray(data, dtype=np.float32).reshape(-1)
    h = np.asarray(rir, dtype=np.float32).reshape(-1)

    if _NC_CACHE is None:
        _NC_CACHE = _build_nc()
    nc = _NC_CACHE

    w = _build_weights(h)
    in_maps = [{"x": _shard(data, i), "w": w} for i in range(NCORES)]
    res = run_bass_kernel_spmd(nc, in_maps, core_ids=list(range(NCORES)))

    y = np.empty(NCORES * BLK_PER_CORE * B, np.float32)
    span = BLK_PER_CORE * B
    for i in range(NCORES):
        y[i * span:(i + 1) * span] = res.results[i]["y"].T.reshape(-1)
    return y[:NOUT]


# revision 33
# speedup vs baseline: 1.5750x; 1.5750x over previous
"""Trainium2 Bass kernel: full 1-D convolution (2,097,152-sample signal with a
32,000-tap RIR) followed by peak-normalization, distributed over 8 NeuronCores.

Algorithm: block-Toeplitz formulation of the convolution.
  y[128q + r] = sum_c sum_s  h[128c + r - s] * x_{q-c}[s]
i.e. y_q = sum_{c=0}^{250} M_c^T x_{q-c}, where M_c[s, r] = h[128c + r - s]
(zero outside [0, 32000)).  Each M_c is a 128x128 Toeplitz matrix built on the
host from the RIR; the x blocks live as columns of an SBUF-resident [128, nblk]
matrix, so the tensor engine runs 251 accumulating matmuls per output tile with
the rhs being a sliding column window of the same SBUF tensor.

Sequence-parallel across 8 cores: core i computes output blocks
[2080 i, 2080 (i+1)) and receives its input span plus a 250-block left halo.
The peak-normalize max is combined with an on-device AllReduce(max) and the
scale (1 / max(m, 1)) is applied before the output leaves the device.

Matmuls run in float32r (TF32-like, full bf16-rate on the PE at free dim >=
256) with fp32 PSUM accumulation.
"""
import numpy as np

B = 128                      # block size / partition count
KLEN = 32_000                # RIR taps
N = 2_097_152                # signal samples
NOUT = N + KLEN - 1          # full-convolution output length
C = 251                      # number of 128-tap Toeplitz chunks
HALO = C - 1                 # left halo in blocks
NCORES = 8
BLK_PER_CORE = 2080          # output blocks per core (8*2080 = 16640 >= 16634)
LOCAL_IN = BLK_PER_CORE + HALO   # input blocks per core (2330)
F = 416                      # matmul moving free dim (psum tile columns)
NTILES = BLK_PER_CORE // F   # 5 psum tiles per core

_NC_CACHE = None


def _build_nc(collective=True, passes=1):
    import concourse.bacc as bacc
    import concourse.bass as bass
    import concourse.mybir as mybir
    from concourse import tile

    f32 = mybir.dt.float32
    f32r = mybir.dt.float32r

    nc = bacc.Bacc("TRN2", target_bir_lowering=False, debug=False,
                   num_devices=NCORES)

    x_in = nc.dram_tensor("x", [B, LOCAL_IN], f32r, kind="ExternalInput")
    w_in = nc.dram_tensor("w", [B, C * B], f32r, kind="ExternalInput")
    y_out = nc.dram_tensor("y", [B, BLK_PER_CORE], f32, kind="ExternalOutput")

    with tile.TileContext(nc) as tc:
        with (
            tc.tile_pool(name="data", bufs=1) as data_pool,
            tc.tile_pool(name="wpool", bufs=1) as wpool,
            tc.tile_pool(name="ps", bufs=1, space="PSUM") as ps_pool,
            tc.tile_pool(name="dram", bufs=1, space="DRAM") as dram_pool,
        ):
            x_sb = data_pool.tile([B, LOCAL_IN], f32r, name="x_sb")
            y_sb = data_pool.tile([B, BLK_PER_CORE], f32, name="y_sb")
            mx = data_pool.tile([B, NTILES], f32, name="mx")
            w_sb = wpool.tile([B, C * B], f32r, name="w_sb")

            # warm the PE (HAM clock gate) during the initial DMA wait with
            # dummy matmuls on a zeroed tile — no data dependencies
            warm = data_pool.tile([B, 512], mybir.dt.bfloat16, name="warm")
            nc.gpsimd.memset(warm[:], 0.0)
            wps = ps_pool.tile([B, 512], f32, name="wps", tag="wps")
            for _ in range(12):
                nc.tensor.matmul(wps[:], warm[:, :B], warm[:, :512],
                                 start=True, stop=True)

            # tile 0 reads x cols [0, HALO+F); load those first (on a separate
            # engine queue from the weights) so the PE can start early.
            XSPLIT = HALO + F
            nc.scalar.dma_start(x_sb[:, :XSPLIT], x_in[:, :XSPLIT])
            # weights in batches, in consumption order (c ascending), small
            # batches first so chunk delivery keeps pace with the tile-0
            # c-loop from the start
            ws = [4, 8, 16]
            while sum(ws) < C:
                ws.append(min(16, C - sum(ws)))
            b0 = 0
            for i, wn in enumerate(ws):
                b1 = b0 + wn
                nc.sync.dma_start(w_sb[:, b0 * B:b1 * B], w_in[:, b0 * B:b1 * B])
                if i == 0:
                    nc.scalar.dma_start(x_sb[:, XSPLIT:], x_in[:, XSPLIT:])
                b0 = b1

            def one_pass():
                # Phase 1, chunk-major over the first C1 chunks: all NTILES
                # psum banks accumulate concurrently, so each weight chunk is
                # consumed NTILES matmuls in a row (~5x134 ns) — faster than
                # HBM delivers chunks (~178 ns), so the PE never stalls on the
                # weight stream.  By chunk C1 the whole weight tensor is
                # resident.  Phase 2, tile-major: tiles finish staggered, so
                # their reduces/copies hide under the next tile's matmuls and
                # only the last tile's reduce lands in the kernel tail.
                C1 = 72
                pss = [ps_pool.tile([B, F], f32, name=f"ps{t}", tag=f"ps{t}")
                       for t in range(NTILES)]

                def mm(t, c):
                    lo = t * F + HALO - c
                    nc.tensor.matmul(
                        pss[t][:],
                        w_sb[:, c * B:(c + 1) * B],
                        x_sb[:, lo:lo + F],
                        start=(c == 0),
                        stop=(c == C - 1),
                    )

                for c in range(C1):
                    for t in range(NTILES):
                        mm(t, c)
                for t in range(NTILES):
                    for c in range(C1, C):
                        mm(t, c)
                    nc.vector.tensor_reduce(
                        mx[:, t:t + 1], pss[t][:], axis=mybir.AxisListType.X,
                        op=mybir.AluOpType.max, apply_absolute_value=True,
                    )
                    if t < NTILES - 1:
                        # hidden under the next tile's phase-2 matmuls
                        nc.vector.tensor_copy(y_sb[:, t * F:(t + 1) * F],
                                              pss[t][:])

                # local scalar max -> all partitions
                am = data_pool.tile([B, 1], f32, name="am")
                nc.vector.tensor_reduce(
                    am[:], mx[:], axis=mybir.AxisListType.X,
                    op=mybir.AluOpType.max,
                )
                gm = data_pool.tile([B, 1], f32, name="gm")
                nc.gpsimd.partition_all_reduce(
                    gm[:], am[:], B, bass.bass_isa.ReduceOp.max
                )

                scb = data_pool.tile([B, 1], f32, name="scb")
                if collective:
                    # global max across the 8 cores
                    cc_in = dram_pool.tile([B, 1], f32, name="cc_in")
                    cc_out = dram_pool.tile([B, 1], f32, name="cc_out",
                                            addr_space="Shared")
                    nc.sync.dma_start(cc_in[:], gm[:])
                    nc.gpsimd.collective_compute(
                        "AllReduce",
                        mybir.AluOpType.max,
                        replica_groups=[list(range(NCORES))],
                        ins=[cc_in[:].opt()],
                        outs=[cc_out[:].opt()],
                    )
                    nc.sync.dma_start(scb[:], cc_out[:])
                else:
                    # single-core variant for TimelineSim (no collectives)
                    nc.vector.tensor_copy(scb[:], gm[:])

                # y *= 1 / max(m, 1); chunked so the store DMAs (on two
                # queues) overlap the scaling
                nc.vector.tensor_scalar_max(scb[:], scb[:], 1.0)
                nc.vector.reciprocal(scb[:], scb[:])
                for t in range(NTILES):
                    sl = slice(t * F, (t + 1) * F)
                    src = pss[t][:] if t == NTILES - 1 else y_sb[:, sl]
                    nc.vector.tensor_scalar_mul(y_sb[:, sl], src, scb[:, 0:1])
                    eng = nc.sync if t % 2 == 0 else nc.scalar
                    eng.dma_start(y_out[:, sl], y_sb[:, sl])

            for _ in range(passes):  # passes > 1 only for wall-clock timing
                one_pass()

    nc.compile()
    return nc


def _build_weights(h):
    """[128, C*128] f32: column block c holds M_c with M_c[s, r] = h[128c+r-s]."""
    hp = np.zeros(B * (C - 1) + 2 * B, np.float32)
    hp[B - 1:B - 1 + KLEN] = h
    idx = (B - 1) + np.arange(B)[None, :] - np.arange(B)[:, None]  # [s, r]
    offs = B * np.arange(C)
    W = hp[offs[:, None, None] + idx[None, :, :]]                  # [C, s, r]
    return np.ascontiguousarray(W.transpose(1, 0, 2).reshape(B, C * B))


def _shard(data, i):
    """Core i's input: blocks [q0-HALO, q0+BLK_PER_CORE) as a [128, LOCAL_IN]
    matrix (column q = block q), zero-padded outside [0, N)."""
    q0 = i * BLK_PER_CORE
    lo = (q0 - HALO) * B
    hi = (q0 + BLK_PER_CORE) * B
    seg = np.zeros(hi - lo, np.float32)
    s0, s1 = max(lo, 0), min(hi, N)
    if s1 > s0:
        seg[s0 - lo:s1 - lo] = data[s0:s1]
    return np.ascontiguousarray(seg.reshape(LOCAL_IN, B).T)


def kernel(data, rir):
    global _NC_CACHE
    from concourse.bass_utils import run_bass_kernel_spmd

    data = np.asarray(data, dtype=np.float32).reshape(-1)
    h = np.asarray(rir, dtype=np.float32).reshape(-1)

    if _NC_CACHE is None:
        _NC_CACHE = _build_nc()
    nc = _NC_CACHE

    w = _build_weights(h)
    in_maps = [{"x": _shard(data, i), "w": w} for i in range(NCORES)]
    res = run_bass_kernel_spmd(nc, in_maps, core_ids=list(range(NCORES)))

    y = np.empty(NCORES * BLK_PER_CORE * B, np.float32)
    span = BLK_PER_CORE * B
    for i in range(NCORES):
        y[i * span:(i + 1) * span] = res.results[i]["y"].T.reshape(-1)
    return y[:NOUT]
